# revision 8
# baseline (speedup 1.0000x reference)
"""MDN-RNN mixture-density loss kernel for Trainium2, SPMD over 8 NeuronCores.

Math (per token row i):
    means/logstds: [K, D] slices of s_mean/s_logstd rows
    z      = (target - mean_k) * exp(-logstd_k)
    logp_k = -0.5 * sum_d z^2 - sum_d logstd_k
    loss   = -mean_i logsumexp_k(log_mix_coeffs + logp_k)

Sharding: data-parallel on the token dim N=16384 -> 2048 rows per core,
no cross-device communication; each core emits nm = -max_k(score) and
S = sum_k exp(score+nm) per row packed [128, 2T]; the host finishes
loss = mean(nm - ln S).

The host uploads bf16 inputs packed per row as [lstd | tgt | mean]
(one contiguous 23.9KB stream per row): HBM traffic halves vs f32 and
each 128-row tile is a single contiguous DMA. bf16 rounding of the
inputs perturbs each row's logsumexp by a zero-mean ~1e-3 relative
amount that averages out over 16384 rows (measured ~1e-4 on the loss).

Engine split per 128-row tile (all big ops bf16 -> DVE 2x mode):
    ACT:    e1 = exp(-lstd), squares w/ accumulate for k=0..2,
            final exp(score+nm) w/ accumulate
    DVE:    diff = target(bcast over k) - mean, z = diff*e1,
            fused square+sum (tensor_tensor_reduce) for k=3..4,
            sls = grouped sum_d logstd, logsumexp smalls
"""

import sys

if "/opt/trn_rl_repo" not in sys.path:
    sys.path.insert(0, "/opt/trn_rl_repo")

import numpy as np

N = 16384
K = 5
D = 1088
KD = K * D
NCORES = 8
R = N // NCORES          # 2048 rows per core
P = 128                  # partitions
T = R // P               # 16 tiles per core

PK = KD + D + KD         # 11968 packed row: [lstd | tgt | mean]
TG0, TG1 = KD, KD + D
MN0 = KD + D

ACT_K = 5                # squares on ACT (k < ACT_K); rest fused on DVE

_NC = None


def _build():
    import concourse.bacc as bacc
    import concourse.bass as bass
    import concourse.tile as tile
    from concourse import mybir

    AF = mybir.ActivationFunctionType
    AL = mybir.AluOpType
    AX = mybir.AxisListType
    f32 = mybir.dt.float32
    bf16 = mybir.dt.bfloat16

    nc = bacc.Bacc("TRN2", debug=False)
    pk = nc.dram_tensor("pk", [R, PK], bf16, kind="ExternalInput").ap()
    lmx = nc.dram_tensor("lmx", [P, T * K], f32, kind="ExternalInput").ap()
    out = nc.dram_tensor("res", [P, 2 * T], f32, kind="ExternalOutput").ap()

    with tile.TileContext(nc) as tc:
        with (
            tc.tile_pool(name="all_p", bufs=4) as all_p,
            tc.tile_pool(name="e1_p", bufs=2) as e1_p,
            tc.tile_pool(name="z_p", bufs=2) as z_p,
            tc.tile_pool(name="small_p", bufs=3) as small_p,
            tc.tile_pool(name="persist", bufs=1) as persist,
        ):
            t_lmx = persist.tile([P, T * K], f32)
            t_nmacc = persist.tile([P, T], f32)   # per-tile -max_k score
            t_sacc = persist.tile([P, T], f32)    # per-tile sum_k exp(score+nm)

            state = {}

            def emit_a(t):
                """Front stage: DMA, e1 = exp(-lstd), diff = tgt - mean."""
                rows = slice(t * P, (t + 1) * P)
                t_all = all_p.tile([P, PK], bf16)
                t_e1 = e1_p.tile([P, KD], bf16)
                if t == 0:
                    # per-k lstd chunks with per-k exp so ACT warms up as
                    # soon as the first 0.28MB lands
                    for k in range(K):
                        nc.sync.dma_start(
                            out=t_all[:, k * D : (k + 1) * D],
                            in_=pk[rows, k * D : (k + 1) * D],
                        )
                    nc.sync.dma_start(out=t_lmx, in_=lmx)
                    nc.sync.dma_start(out=t_all[:, KD:PK], in_=pk[rows, KD:PK])
                    for k in range(K):
                        nc.scalar.activation(
                            out=t_e1[:, k * D : (k + 1) * D],
                            in_=t_all[:, k * D : (k + 1) * D],
                            func=AF.Exp, scale=-1.0,
                        )
                else:
                    nc.sync.dma_start(out=t_all, in_=pk[rows])
                    nc.scalar.activation(
                        out=t_e1, in_=t_all[:, 0:KD], func=AF.Exp, scale=-1.0
                    )

                # diff = target (broadcast over k) - mean (3D views, all bf16)
                t_tg = t_all[:, TG0:TG1]
                tgt_b = bass.AP(
                    tensor=t_tg.tensor, offset=t_tg.offset,
                    ap=[t_tg.ap[0], [0, K], t_tg.ap[1]],
                )
                mean3 = t_all[:, MN0:PK].rearrange("p (k d) -> p k d", k=K)
                t_z = z_p.tile([P, K, D], bf16)
                nc.vector.tensor_tensor(out=t_z, in0=tgt_b, in1=mean3, op=AL.subtract)
                state[t] = (t_all, t_e1, t_z)

            def emit_b(t):
                """Back stage: z, per-k sum z^2, sls, logsumexp smalls."""
                t_all, t_e1, t_z = state.pop(t)
                e13 = t_e1.rearrange("p (k d) -> p k d", k=K)
                # z = diff * e1 (bf16 2x mode): k=0..3 on DVE, k=4 on Pool
                nc.vector.tensor_tensor(
                    out=t_z[:, 0:4, :], in0=t_z[:, 0:4, :], in1=e13[:, 0:4, :],
                    op=AL.mult,
                )
                nc.gpsimd.tensor_tensor(
                    out=t_z[:, 4, :], in0=t_z[:, 4, :], in1=e13[:, 4, :],
                    op=AL.mult,
                )

                t_h = small_p.tile([P, K], f32)
                # ACT squares w/ accumulate
                for k in range(K):
                    nc.scalar.activation(
                        out=t_z[:, k, :], in_=t_z[:, k, :], func=AF.Square,
                        accum_out=t_h[:, k : k + 1],
                    )
                # sls_k = sum_d logstd (grouped 3D reduce, bf16 out for 2x)
                t_sls = small_p.tile([P, K], bf16)
                lstd3 = t_all[:, 0:KD].rearrange("p (k d) -> p k d", k=K)
                with nc.allow_low_precision(reason="sls |err| ~0.1 on N(0,33), fine"):
                    nc.vector.tensor_reduce(
                        out=t_sls, in_=lstd3, axis=AX.X, op=AL.add
                    )

                # score_k = -0.5*h_k - sls_k + lmx_k ; nm = -max_k score
                t_q = small_p.tile([P, K], f32)
                nc.vector.scalar_tensor_tensor(
                    out=t_q, in0=t_h, scalar=-0.5, in1=t_sls,
                    op0=AL.mult, op1=AL.subtract,
                )
                t_c = small_p.tile([P, K], f32)
                nc.vector.tensor_tensor(
                    out=t_c, in0=t_q, in1=t_lmx[:, t * K : (t + 1) * K], op=AL.add
                )
                nc.vector.tensor_reduce(
                    out=t_nmacc[:, t : t + 1], in_=t_c, axis=AX.X, op=AL.max, negate=True
                )
                # S_t = sum_k exp(score + nm)
                t_e = small_p.tile([P, K], f32)
                nc.scalar.activation(
                    out=t_e, in_=t_c, func=AF.Exp, bias=t_nmacc[:, t : t + 1],
                    scale=1.0, accum_out=t_sacc[:, t : t + 1],
                )

            # software-pipelined emission: tile t+1's front stage is queued
            # before tile t's back stage so ACT's exp(t+1) overlaps DVE's
            # mult/reduce chain of tile t
            emit_a(0)
            for t in range(T):
                if t + 1 < T:
                    emit_a(t + 1)
                emit_b(t)

            # ship nm and S; the host finishes loss = mean(nm - ln S)
            nc.sync.dma_start(out=out[:, 0:T], in_=t_nmacc)
            nc.sync.dma_start(out=out[:, T : 2 * T], in_=t_sacc)

    nc.compile()
    return nc


def get_nc():
    global _NC
    if _NC is None:
        _NC = _build()
    return _NC


def make_in_maps(target, s_mean, s_logstd, log_mix_coeffs):
    import ml_dtypes

    BF = ml_dtypes.bfloat16
    tb = np.asarray(target, dtype=np.float32).astype(BF)
    mb = np.asarray(s_mean, dtype=np.float32).astype(BF)
    lb = np.asarray(s_logstd, dtype=np.float32).astype(BF)
    lm = np.ascontiguousarray(np.asarray(log_mix_coeffs, dtype=np.float32))
    in_maps = []
    for c in range(NCORES):
        rows = slice(c * R, (c + 1) * R)
        pk = np.empty((R, PK), dtype=BF)
        pk[:, 0:KD] = lb[rows]
        pk[:, TG0:TG1] = tb[rows]
        pk[:, MN0:PK] = mb[rows]
        # pack log-mix so tile t's [128, K] block sits at columns [t*K, (t+1)*K)
        lmx = lm[rows].reshape(T, P, K).transpose(1, 0, 2).reshape(P, T * K)
        in_maps.append({
            "pk": pk,
            "lmx": np.ascontiguousarray(lmx),
        })
    return in_maps


def combine(results):
    # res[:, :T] = nm = -max_k score ; res[:, T:] = S = sum_k exp(score+nm)
    # lse = -nm + ln(S); loss = -mean(lse) = mean(nm - ln(S))
    total = 0.0
    for r in results:
        res = np.asarray(r["res"], dtype=np.float64)
        nm, s = res[:, :T], res[:, T:]
        total += float((nm - np.log(s)).sum())
    return np.float32(total / N)


def kernel(target, s_mean, s_logstd, log_mix_coeffs):
    from concourse.bass_utils import run_bass_kernel_spmd

    nc = get_nc()
    in_maps = make_in_maps(target, s_mean, s_logstd, log_mix_coeffs)
    res = run_bass_kernel_spmd(nc, in_maps, core_ids=list(range(NCORES)))
    return combine(res.results)
